# revision 6
# baseline (speedup 1.0000x reference)
"""GATv2 attention layer on 8 trn2 cores — v4.

Changes vs baseline (all exact, same math):
  * Vq = exp(0.2 ej) folded into the TS scalars (2-ptr TS costs the same
    as 1-ptr, measured): pm = adj * max(Ti*exp(ej), exp(0.2 ej)).
  * lhsT blocks [0.25*h_h | 1] come straight out of the h-matmul: W is
    augmented with a zero column per head and a K=1 ones-row matmul adds
    the 1s, so evacuation is ONE contiguous [P,260] copy per chunk on ACT
    (was 128 per-(j,h) scaled copies, ~40us of ACT).
  * mask multiply batched JG=4 (FD=8192).
  * NACT chunks offloaded: ACT computes r = relu(exp(ej)*Ti - Vq) per
    head; DVE only does the mask TT (adj*r); the missing adj*Vq*rhs term
    is added by an extra PE matmul stream (Vq-scaled lhsT vs adjT).
    These chunks' matmuls issue last so the PE never stalls on ACT.

Sharding: core c owns destination rows i in [512c, 512c+512); W/x
replicated; adj fed pre-transposed+sliced per core.
"""

import contextlib
import os
import sys

import numpy as np

for _p in ("/opt/trn_rl_repo", "/root/.axon_site/_ro/trn_rl_repo"):
    if os.path.isdir(_p) and _p not in sys.path:
        sys.path.append(_p)

import concourse.bass as bass
import concourse.mybir as mybir
from concourse import bacc
import concourse.tile as tile
from concourse.tile import add_dep_helper
from concourse.bass_utils import run_bass_kernel_spmd
from concourse.masks import make_identity

N = 4096
F_IN = 256
HEADS = 4
F_OUT = 64
CORES = 8
I_PER_CORE = N // CORES          # 512
P = 128
NJC = N // P                     # 32 j-chunks
NIC = I_PER_CORE // P            # 4 i-chunks
ICOL = F_OUT + 1                 # 65: [0.25*h | ones]
RCOLS = HEADS * ICOL             # 260
KCH = 2
JG = 4                           # j-chunks per batched mask multiply

F32 = mybir.dt.float32
F16 = mybir.dt.float16

_BASS = None
LAST_RESULT = None

NACT = 12                        # trailing chunks offloaded to ACT


def _build(reps=1, nact=NACT):
    nc = bacc.Bacc()
    xT_d = nc.dram_tensor("xT", [F_IN, N], F16, kind="ExternalInput")
    xo_d = nc.dram_tensor("xo", [1, N], F16, kind="ExternalInput")
    W_d = nc.dram_tensor("Wa", [F_IN, RCOLS], F16, kind="ExternalInput")
    wo_d = nc.dram_tensor("wo", [1, RCOLS], F16, kind="ExternalInput")
    tib_d = nc.dram_tensor("tib", [P, HEADS, I_PER_CORE], F16,
                           kind="ExternalInput")
    ejr_d = nc.dram_tensor("ejr", [P, NJC, HEADS], F32, kind="ExternalInput")
    adjT_d = nc.dram_tensor("adjT", [N, I_PER_CORE], F16, kind="ExternalInput")
    out_d = nc.dram_tensor("out", [I_PER_CORE, F_OUT], F32,
                           kind="ExternalOutput")

    EXP = mybir.ActivationFunctionType.Exp
    RELU = mybir.ActivationFunctionType.Relu
    CPY = mybir.ActivationFunctionType.Copy
    MULT = mybir.AluOpType.mult
    MAX = mybir.AluOpType.max
    ADD = mybir.AluOpType.add

    n_dve = NJC - nact
    with tile.TileContext(nc) as tc:
        with (
            tc.tile_pool(name="cst", bufs=1) as cst,
            tc.tile_pool(name="adj", bufs=1) as adjp,
            tc.tile_pool(name="q4p", bufs=1) as q4p,
            tc.tile_pool(name="pm4p", bufs=2) as pm4p,
            tc.tile_pool(name="qo", bufs=3) as qop,
            tc.tile_pool(name="fin", bufs=2) as fin,
            tc.tile_pool(name="ps1", bufs=2, space="PSUM") as ps1,
            tc.tile_pool(name="psacc", bufs=1, space="PSUM") as psacc,
            (tc.For_i(0, reps, 1) if reps > 1 else contextlib.nullcontext()),
        ):
            # ---- loads -------------------------------------------------
            # tib/ejr + first adj chunks first so the DVE score pipeline
            # starts immediately; bulk x/W loads follow.
            tib = cst.tile([P, HEADS, I_PER_CORE], F16, tag="tib")
            nc.sync.dma_start(tib[:], tib_d[:])
            ejr = cst.tile([P, NJC, HEADS], F32, tag="ejr")
            nc.sync.dma_start(ejr[:], ejr_d[:])
            adj_sb = adjp.tile([P, NJC, I_PER_CORE], F16, tag="adj_sb")
            for j in range(8):
                nc.sync.dma_start(adj_sb[:, j, :], adjT_d[j * P:(j + 1) * P, :])
            xk = [cst.tile([P, N], F16, name=f"xk{k}", tag=f"xk{k}")
                  for k in range(KCH)]
            wk = [cst.tile([P, RCOLS], F16, name=f"wk{k}", tag=f"wk{k}")
                  for k in range(KCH)]
            for k in range(KCH):
                nc.sync.dma_start(wk[k][:], W_d[k * P:(k + 1) * P, :])
                for q_ in range(4):
                    nc.sync.dma_start(
                        xk[k][:, q_ * (N // 4):(q_ + 1) * (N // 4)],
                        xT_d[k * P:(k + 1) * P,
                             q_ * (N // 4):(q_ + 1) * (N // 4)])
            xo = cst.tile([1, N], F16, tag="xo")
            nc.sync.dma_start(xo[:], xo_d[:])
            wo = cst.tile([1, RCOLS], F16, tag="wo")
            nc.sync.dma_start(wo[:], wo_d[:])
            for j in range(8, NJC):
                nc.sync.dma_start(adj_sb[:, j, :], adjT_d[j * P:(j + 1) * P, :])

            # node vectors: se = exp(ej), vq = exp(0.2 ej), vqn = -vq
            se = cst.tile([P, NJC, HEADS], F32, tag="se")
            vq = cst.tile([P, NJC, HEADS], F32, tag="vq")
            vqn = cst.tile([P, NJC, HEADS], F32, tag="vqn")
            nc.scalar.activation(se[:], ejr[:], EXP, scale=1.0)
            nc.scalar.activation(vq[:], ejr[:], EXP, scale=0.2)
            nc.vector.tensor_scalar(vqn[:], vq[:], -1.0, None, op0=MULT)

            # hoist the first ACT chunks' relus into the otherwise-idle
            # ACT window while the PE runs the h matmuls; qa is static so
            # later relus never wait on buffer rotation
            NHOIST = min(4, nact)
            qa = None
            if nact:
                qa = cst.tile([P, nact, HEADS, I_PER_CORE], F16,
                              name="qa", tag="qa")
            for jj in range(NHOIST):
                j = (NJC - nact) + jj
                for h in range(HEADS):
                    nc.scalar.activation(
                        qa[:, jj, h, :], tib[:, h, :], RELU,
                        bias=vqn[:, j, h:h + 1], scale=se[:, j, h:h + 1])

            # ---- phase 1: rhs[:, j] = [0.25*h_h | 1]*4 via W_aug -------
            rhs = cst.tile([P, NJC, RCOLS], F16, tag="rhs")
            for j in range(NJC):
                pt = ps1.tile([P, RCOLS], F32, tag="ps_h")
                nc.tensor.matmul(pt[:], xk[0][:, j * P:(j + 1) * P], wk[0][:],
                                 start=True, stop=False)
                nc.tensor.matmul(pt[:], xk[1][:, j * P:(j + 1) * P], wk[1][:],
                                 start=False, stop=False)
                nc.tensor.matmul(pt[:], xo[:, j * P:(j + 1) * P], wo[:],
                                 start=False, stop=True)
                nc.scalar.copy(rhs[:, j, :], pt[:])

            if nact:
                rhsv = cst.tile([P, nact, HEADS, ICOL], F16, tag="rhsv")

            # ---- phase 3: stream the score matrix ---------------------
            junk = fin.tile([P, 2], F32, tag="junk")
            pt1 = nc.vector.tensor_copy(junk[:, 0:1], tib[:, 0, 0:1])
            pt2 = nc.vector.tensor_copy(junk[:, 1:2], se[:, 0, 0:1])

            ident = cst.tile([P, P], F32, tag="ident")
            make_identity(nc, ident[:])

            ot = [fin.tile([P, F_OUT], F32, name=f"ot{ic}", tag=f"ot{ic}")
                  for ic in range(NIC)]
            acc = [psacc.tile([ICOL, I_PER_CORE], F32, name=f"acc{h}",
                              tag=f"acc{h}") for h in range(HEADS)]

            mm_left = [NJC + nact] * HEADS   # MMs per acc bank
            n_mm = [0] * HEADS

            def agg_mm(h, lhs_ap, mov_ap):
                nc.tensor.matmul(acc[h][:], lhs_ap, mov_ap,
                                 start=(n_mm[h] == 0),
                                 stop=(n_mm[h] == mm_left[h] - 1))
                n_mm[h] += 1

            # DVE chunks in JG groups: TS then one batched mask multiply
            for g in range(0, n_dve, JG):
                pair = range(g, min(g + JG, n_dve))
                npair = len(pair)
                q4 = q4p.tile([P, JG, HEADS, I_PER_CORE], F16, tag="q4")
                pm4 = pm4p.tile([P, JG, HEADS, I_PER_CORE], F16, tag="pm4")
                for jj, j in enumerate(pair):
                    for h in range(HEADS):
                        qi = nc.vector.tensor_scalar(
                            q4[:, jj, h, :], tib[:, h, :],
                            se[:, j, h:h + 1], vq[:, j, h:h + 1],
                            op0=MULT, op1=MAX)
                        if g == 0 and jj == 0 and h == 0:
                            add_dep_helper(qi.ins, pt1.ins, sync=False,
                                           reason="pretouch order")
                            add_dep_helper(qi.ins, pt2.ins, sync=False,
                                           reason="pretouch order")
                adj_b = adj_sb[:, g:g + npair, :].unsqueeze(2).broadcast_to(
                    [P, npair, HEADS, I_PER_CORE])
                nc.vector.tensor_tensor(pm4[:, 0:npair], q4[:, 0:npair],
                                        adj_b, op=MULT)
                for jj, j in enumerate(pair):
                    for h in range(HEADS):
                        agg_mm(h, rhs[:, j, h * ICOL:(h + 1) * ICOL],
                               pm4[:, jj, h, :])

            # ACT chunks: relu on ACT, mask TT on DVE, Vq term on PE.
            # rhsv copies interleave per chunk so ACT's stream stays smooth.
            for jj in range(nact):
                j = n_dve + jj
                if jj >= NHOIST:
                    for h in range(HEADS):
                        nc.scalar.activation(
                            qa[:, jj, h, :], tib[:, h, :], RELU,
                            bias=vqn[:, j, h:h + 1], scale=se[:, j, h:h + 1])
                pm4 = qop.tile([P, HEADS, I_PER_CORE], F16, tag="pma")
                for h in range(HEADS):
                    nc.scalar.activation(
                        rhsv[:, jj, h, :], rhs[:, j, h * ICOL:(h + 1) * ICOL],
                        CPY, scale=vq[:, j, h:h + 1])
                adj_b = adj_sb[:, j, :].unsqueeze(1).broadcast_to(
                    [P, HEADS, I_PER_CORE])
                nc.vector.tensor_tensor(pm4[:], qa[:, jj], adj_b, op=MULT)
                for h in range(HEADS):
                    agg_mm(h, rhsv[:, jj, h, :], adj_sb[:, j, :])
                    agg_mm(h, rhs[:, j, h * ICOL:(h + 1) * ICOL],
                           pm4[:, h, :])

            # finalize: evacuate, transpose back to [i, f], normalize, mean
            for h in range(HEADS):
                numt = fin.tile([ICOL, I_PER_CORE], F32, name=f"numt{h}",
                                tag="numt")
                nc.scalar.copy(numt[:], acc[h][:])
                for ic in range(NIC):
                    ps_t = ps1.tile([P, ICOL], F32, tag="ps_t")
                    nc.tensor.matmul(ps_t[:], numt[:, ic * P:(ic + 1) * P],
                                     ident[:ICOL, :ICOL], is_transpose=True,
                                     start=True, stop=True)
                    rec = fin.tile([P, 1], F32, tag="rec")
                    nc.vector.reciprocal(rec[:], ps_t[:, F_OUT:F_OUT + 1])
                    if h == 0:
                        nc.scalar.activation(ot[ic][:], ps_t[:, :F_OUT],
                                             CPY, scale=rec[:])
                    else:
                        nc.vector.scalar_tensor_tensor(
                            ot[ic][:], ps_t[:, :F_OUT], rec[:], ot[ic][:],
                            op0=MULT, op1=ADD)

            for ic in range(NIC):
                nc.sync.dma_start(out_d[ic * P:(ic + 1) * P, :], ot[ic][:])

    nc.finalize()
    return nc


def _host_prep(x, adj, W, a):
    x = np.asarray(x, np.float32)
    adj = np.asarray(adj)
    W = np.asarray(W, np.float32)
    a = np.asarray(a, np.float32).reshape(-1)
    a1, a2 = a[:F_OUT], a[F_OUT:]

    w1 = np.stack([W[:, 64 * h:64 * h + 64] @ a1 for h in range(HEADS)], 1)
    w2 = np.stack([W[:, 64 * h:64 * h + 64] @ a2 for h in range(HEADS)], 1)
    ei = x @ w1                                   # [N, H] f32
    ej = x @ w2                                   # [N, H] f32
    ti16 = np.minimum(np.exp(0.8 * ei), 6e4).astype(np.float16)   # [N, H]

    xT = np.ascontiguousarray(x.T.astype(np.float16))
    Wa = np.zeros((F_IN, RCOLS), np.float16)
    wo = np.zeros((1, RCOLS), np.float16)
    for h in range(HEADS):
        Wa[:, h * ICOL:h * ICOL + F_OUT] = 0.25 * W[:, 64 * h:64 * h + 64]
        wo[0, h * ICOL + F_OUT] = 1.0
    ejr = np.ascontiguousarray(ej.reshape(NJC, P, HEADS).transpose(1, 0, 2))
    adjT = adj.T.astype(np.float16)               # [j, i]

    in_maps = []
    for c in range(CORES):
        sl = slice(c * I_PER_CORE, (c + 1) * I_PER_CORE)
        in_maps.append({
            "xT": xT,
            "xo": np.ones((1, N), np.float16),
            "Wa": Wa,
            "wo": wo,
            "tib": np.ascontiguousarray(
                np.broadcast_to(ti16[sl].T[None], (P, HEADS, I_PER_CORE))),
            "ejr": ejr,
            "adjT": np.ascontiguousarray(adjT[:, sl]),
        })
    return in_maps


def kernel(x, adj, W, a):
    global _BASS, LAST_RESULT
    if _BASS is None:
        _BASS = _build()
    in_maps = _host_prep(x, adj, W, a)
    res = run_bass_kernel_spmd(_BASS, in_maps, core_ids=list(range(CORES)))
    LAST_RESULT = res
    return np.concatenate([res.results[c]["out"] for c in range(CORES)],
                          axis=0)
